# revision 1
# baseline (speedup 1.0000x reference)
"""AudioDecoder (2-layer LSTM, teacher forcing) Bass/Tile kernel for TRN2 — v2.

Data-parallel over batch (4096 -> 8 cores x 512), weights replicated.
Feature-major layout: [D, B] tensors stored as SBUF tiles [128, n*512].

v2 vs baseline:
 - L0 bias folded into the ih0 matmul via a ones-row (contraction 80 -> 81),
   enabling merged [128, 1024] L0 gate activations (4 insts instead of 8).
 - Gate activations output fp16 (except f: fp32) -> DVE 2x mode on i*g and
   o*tanh(c); c stays fp32; f*c runs on GPSIMD off the critical path.
 - Gate order f,i,g,o so the Pool f*c mul starts earliest.
 - Chain-independent matmuls (fc, x-transpose, hh1) emitted ahead of the
   chain-dependent hh0/ih1 within each iteration.
"""

import numpy as np
from contextlib import ExitStack

import concourse.bass as bass
import concourse.tile as tile
from concourse import bacc
from concourse import mybir
from concourse.masks import make_identity

F32 = mybir.dt.float32
F32R = mybir.dt.float32r
FP16 = mybir.dt.float16
MM_DT = FP16
AF = mybir.ActivationFunctionType

LATENT = 128
FEAT = 80
HID = 256
G = 4 * HID  # 1024
BL = 512     # per-core batch
NMB = BL // 128  # 4 batch chunks of 128
MAXT = 100

# gate order in PyTorch layout: i(0), f(1), g(2), o(3); chunk c -> gate c//2.
# processing order: f, i, g, o
GATE_SEQ = ((1, AF.Sigmoid), (0, AF.Sigmoid), (2, AF.Tanh), (3, AF.Sigmoid))


def build_kernel(T=MAXT, TC=10, TF=4, reps=1):
    nc = bacc.Bacc()

    z = nc.dram_tensor("z", [BL, LATENT], F32, kind="ExternalInput")
    tseq = nc.dram_tensor("target_seq", [BL, MAXT, FEAT], F32, kind="ExternalInput")
    W_li = nc.dram_tensor("W_li", [FEAT, LATENT], F32, kind="ExternalInput")
    b_li = nc.dram_tensor("b_li", [FEAT], F32, kind="ExternalInput")
    W_ih0 = nc.dram_tensor("W_ih0", [G, FEAT], F32, kind="ExternalInput")
    W_hh0 = nc.dram_tensor("W_hh0", [G, HID], F32, kind="ExternalInput")
    b_ih0 = nc.dram_tensor("b_ih0", [G], F32, kind="ExternalInput")
    b_hh0 = nc.dram_tensor("b_hh0", [G], F32, kind="ExternalInput")
    W_ih1 = nc.dram_tensor("W_ih1", [G, HID], F32, kind="ExternalInput")
    W_hh1 = nc.dram_tensor("W_hh1", [G, HID], F32, kind="ExternalInput")
    b_ih1 = nc.dram_tensor("b_ih1", [G], F32, kind="ExternalInput")
    b_hh1 = nc.dram_tensor("b_hh1", [G], F32, kind="ExternalInput")
    W_fc = nc.dram_tensor("W_fc", [FEAT, HID], F32, kind="ExternalInput")
    b_fc = nc.dram_tensor("b_fc", [FEAT], F32, kind="ExternalInput")
    out = nc.dram_tensor("out", [BL, MAXT, FEAT], F32, kind="ExternalOutput")

    with TileKernel(nc, T, TC, TF, reps) as k:
        k.run(z, tseq, W_li, b_li, W_ih0, W_hh0, b_ih0, b_hh0,
              W_ih1, W_hh1, b_ih1, b_hh1, W_fc, b_fc, out)
    nc.compile()
    return nc


class TileKernel:
    def __init__(self, nc, T, TC, TF, reps=1):
        self.nc = nc
        self.T, self.TC, self.TF = T, TC, TF
        self.reps = reps
        self.ctx = ExitStack()

    def __enter__(self):
        self.tc = self.ctx.enter_context(tile.TileContext(self.nc))
        return self

    def __exit__(self, *a):
        return self.ctx.__exit__(*a)

    def run(self, z, tseq, W_li, b_li, W_ih0, W_hh0, b_ih0, b_hh0,
            W_ih1, W_hh1, b_ih1, b_hh1, W_fc, b_fc, out):
        nc, tc, ctx = self.nc, self.tc, self.ctx
        T, TC, TF = self.T, self.TC, self.TF

        const = ctx.enter_context(tc.tile_pool(name="const", bufs=1))

        ident0 = const.tile([128, 128], F32, tag="ident0")
        make_identity(nc, ident0)
        ident = const.tile([128, 128], F32R, tag="ident")
        nc.scalar.activation(ident[:], ident0[:], AF.Copy)
        ident16 = const.tile([128, 128], MM_DT, tag="ident16")
        nc.scalar.activation(ident16[:], ident0[:], AF.Copy)

        # ---------------- weight prep (transpose to lhsT layouts) -----------
        # wt_ih0 [81, 8*128]: chunk c = W_ih0[c*128:(c+1)*128, :].T; row 80 = b0 chunk
        wt_ih0 = const.tile([FEAT + 1, 8 * 128], MM_DT, tag="wt_ih0")
        wt_hh0 = const.tile([128, 16 * 128], MM_DT, tag="wt_hh0")
        wt_ih1 = const.tile([128, 16 * 128], MM_DT, tag="wt_ih1")
        wt_hh1 = const.tile([128, 16 * 128], MM_DT, tag="wt_hh1")
        wfc_mov = const.tile([128, 2 * FEAT], MM_DT, tag="wfc_mov")
        wli_t = const.tile([128, FEAT], MM_DT, tag="wli_t")
        b1_sb = const.tile([128, 8], F32, tag="b1_sb")
        bli_sb = const.tile([FEAT, 1], F32, tag="bli_sb")
        bfc4 = const.tile([128, NMB * FEAT], F32, tag="bfc4")
        zt_sb = const.tile([128, BL], MM_DT, tag="zt_sb")

        with tc.tile_pool(name="setup", bufs=2) as setup, \
             tc.tile_pool(name="setup_ps", bufs=4, space="PSUM") as sps:

            def transpose_to(dst_ap, src_ap):
                # dst[f, p] = src[p, f]; fp32r matmul src.T @ I (even N>=2).
                p, fr = src_ap.shape
                n2 = 2 if p == 1 else p
                pst = sps.tile([128, 128], F32, tag="tps")
                nc.tensor.matmul(pst[:fr, :n2], src_ap, ident[:p, :n2],
                                 start=True, stop=True)
                nc.scalar.activation(dst_ap, pst[:fr, :p], AF.Copy)

            # W_ih0 [1024, 80] -> wt_ih0 rows 0..79
            wn = setup.tile([128, 8, FEAT], F32R, tag="wn_ih0")
            nc.sync.dma_start(wn[:], W_ih0[:, :].rearrange("(j p) f -> p j f", p=128).bitcast(F32R))
            for j in range(8):
                transpose_to(wt_ih0[0:FEAT, j * 128:(j + 1) * 128], wn[:, j, :])

            # W_hh0 / W_ih1 / W_hh1 [1024, 256]
            for W, wt, tg in ((W_hh0, wt_hh0, "wn_hh0"), (W_ih1, wt_ih1, "wn_ih1"),
                              (W_hh1, wt_hh1, "wn_hh1")):
                wn = setup.tile([128, 8, HID], F32R, tag=tg)
                nc.sync.dma_start(wn[:], W[:, :].rearrange("(j p) f -> p j f", p=128).bitcast(F32R))
                for j in range(8):
                    for kk in range(2):
                        transpose_to(wt[:, (kk * 8 + j) * 128:(kk * 8 + j + 1) * 128],
                                     wn[:, j, kk * 128:(kk + 1) * 128])

            # W_fc [80, 256] -> moving rhs [256, 80] chunks
            wn = setup.tile([FEAT, HID], F32R, tag="wn_fc")
            nc.sync.dma_start(wn[:], W_fc[:, :].bitcast(F32R))
            for kk in range(2):
                transpose_to(wfc_mov[:, kk * FEAT:(kk + 1) * FEAT],
                             wn[:, kk * 128:(kk + 1) * 128])

            # W_li [80, 128] -> [128, 80]
            wn = setup.tile([FEAT, LATENT], F32R, tag="wn_li")
            nc.sync.dma_start(wn[:], W_li[:, :].bitcast(F32R))
            transpose_to(wli_t[:, :], wn[:, :])

            # biases: b0 = b_ih0 + b_hh0 -> fp16 row 80 of wt_ih0 (per chunk).
            #         b1 = b_ih1 + b_hh1 -> [128, 8] per-chunk columns.
            ta0 = setup.tile([1, G], F32, tag="b0a")
            tb0 = setup.tile([1, G], F32, tag="b0b")
            nc.sync.dma_start(ta0[:], b_ih0[None, :])
            nc.sync.dma_start(tb0[:], b_hh0[None, :])
            b0row = setup.tile([1, G], MM_DT, tag="b0s")
            nc.vector.tensor_add(b0row[:], ta0[:], tb0[:])
            nc.sync.dma_start(wt_ih0[FEAT:FEAT + 1, :], b0row[:, :])

            ta1 = setup.tile([1, G], F32, tag="b1a")
            tb1 = setup.tile([1, G], F32, tag="b1b")
            nc.sync.dma_start(ta1[:], b_ih1[None, :])
            nc.sync.dma_start(tb1[:], b_hh1[None, :])
            tsum1 = setup.tile([1, G], F32R, tag="b1s")
            nc.vector.tensor_add(tsum1[:], ta1[:], tb1[:])
            for j in range(8):
                transpose_to(b1_sb[:, j:j + 1], tsum1[:, j * 128:(j + 1) * 128])

            tb = setup.tile([1, FEAT], F32R, tag="bli")
            nc.sync.dma_start(tb[:], b_li[None, :].bitcast(F32R))
            transpose_to(bli_sb[:, :], tb[:, :])

            # b_fc broadcast to [128, 4*80]
            bfc_ap = b_fc[:]
            bfc_b = bass.AP(tensor=bfc_ap.tensor, offset=bfc_ap.offset,
                            ap=[[0, 128], [1, FEAT]])
            nc.gpsimd.dma_start(bfc4[:, 0:FEAT], bfc_b)
            for mb in range(1, NMB):
                nc.vector.tensor_copy(bfc4[:, mb * FEAT:(mb + 1) * FEAT],
                                      bfc4[:, 0:FEAT])

            # z.T [128, 512]
            zn = setup.tile([128, NMB, LATENT], F32R, tag="zn")
            nc.sync.dma_start(zn[:], z[:, :].rearrange("(mb p) l -> p mb l", p=128).bitcast(F32R))
            zps = sps.tile([128, BL], F32, tag="zps")
            for mb in range(NMB):
                nc.tensor.matmul(zps[:, mb * 128:(mb + 1) * 128], zn[:, mb, :],
                                 ident[:, :], start=True, stop=True)
            nc.scalar.activation(zt_sb[:], zps[:], AF.Copy)

        # ---------------- pools for the time loop ---------------------------
        pg0 = ctx.enter_context(tc.tile_pool(name="pg0", bufs=2, space="PSUM"))
        pg1 = ctx.enter_context(tc.tile_pool(name="pg1", bufs=3, space="PSUM"))
        pmisc = ctx.enter_context(tc.tile_pool(name="pmisc", bufs=1, space="PSUM"))
        acts = ctx.enter_context(tc.tile_pool(name="acts", bufs=3))
        tmp = ctx.enter_context(tc.tile_pool(name="tmp", bufs=3))
        states = ctx.enter_context(tc.tile_pool(name="states", bufs=2))
        xpool = ctx.enter_context(tc.tile_pool(name="xpool", bufs=3))
        xin = ctx.enter_context(tc.tile_pool(name="xin", bufs=3))
        ypool = ctx.enter_context(tc.tile_pool(name="ypool", bufs=2))

        ones_row = const.tile([1, BL], MM_DT, tag="ones_row")
        nc.vector.memset(ones_row[:], 1.0)
        # step-0 xT lives in its own const tile (rows 0:80 from W_li@z, row 80 = 1)
        xT0 = const.tile([FEAT + 1, BL], MM_DT, tag="xT0")
        nc.sync.dma_start(xT0[FEAT:FEAT + 1, :], ones_row[:, :])

        tc.strict_bb_all_engine_barrier()

        for rep in range(self.reps):
            if rep:
                tc.strict_bb_all_engine_barrier()
            h0_last = c0_last = h1_last = c1_last = None

            # x0.T = W_li @ z.T + b_li  (into the step-0 xT slot rows 0..79)
            x0ps = pmisc.tile([128, BL], F32, tag="misc", name="x0ps")
            nc.tensor.matmul(x0ps[0:FEAT, :], wli_t[:], zt_sb[:], start=True, stop=True)
            nc.scalar.activation(xT0[0:FEAT, :], x0ps[0:FEAT, :], AF.Identity,
                                 bias=bli_sb[:, 0:1])
            xT = xT0

            # input chunks: x for step t (1..T-1) is tseq[:, t-1, :].
            n_chunks = (T - 1 + TC - 1) // TC if T > 1 else 0
            xchunks = []
            for c in range(n_chunks):
                tlo = c * TC
                thi = min((c + 1) * TC, T - 1)
                xst = xin.tile([128, NMB, TC, FEAT], F32, tag="xst", bufs=3)
                nc.sync.dma_start(
                    xst[:, :, 0:thi - tlo, :],
                    tseq[:, tlo:thi, :].rearrange("(mb p) t f -> p mb t f", p=128))
                xc = xin.tile([128, NMB, TC, FEAT + 1], MM_DT, tag="xc")
                nc.vector.tensor_copy(xc[:, :, 0:thi - tlo, 0:FEAT],
                                      xst[:, :, 0:thi - tlo, :])
                nc.vector.memset(xc[:, :, 0:thi - tlo, FEAT:FEAT + 1], 1.0)
                xchunks.append(xc)

            ystage = None
            h1_hist = {}

            for it in range(T + 2):
                st1 = it - 1   # layer-1 step handled this iteration
                st2 = it - 2   # fc/output step handled this iteration

                # ---- fc + output for st2 (chain-independent: h1(st2) old) ----
                if 0 <= st2 < T:
                    if st2 % TF == 0:
                        ystage = ypool.tile([128, NMB, TF, FEAT], F32, tag="ystage")
                    yps = pmisc.tile([128, BL], F32, tag="misc", name=f"yps_{it}_{rep}")
                    h1_fc = h1_hist.pop(st2)
                    for mb in range(NMB):
                        for kk in range(2):
                            nc.tensor.matmul(
                                yps[:, mb * FEAT:(mb + 1) * FEAT],
                                h1_fc[:, kk * BL + mb * 128:kk * BL + (mb + 1) * 128],
                                wfc_mov[:, kk * FEAT:(kk + 1) * FEAT],
                                start=(kk == 0), stop=(kk == 1))
                    nc.vector.tensor_add(
                        ystage[:, :, st2 % TF, :],
                        yps[:, 0:NMB * FEAT].rearrange("p (mb f) -> p mb f", mb=NMB),
                        bfc4[:].rearrange("p (mb f) -> p mb f", mb=NMB))
                    if st2 % TF == TF - 1 or st2 == T - 1:
                        t0 = (st2 // TF) * TF
                        nf = st2 - t0 + 1
                        out_r = out[:, :, :].rearrange("(mb p) t f -> mb p t f", p=128)
                        for mb in range(NMB):
                            nc.sync.dma_start(out_r[mb, :, t0:t0 + nf, :],
                                              ystage[:, mb, 0:nf, :])

                # ---- x-transpose for step it+1 (chain-independent) ----
                if it + 1 < T:
                    c_idx, slot = it // TC, it % TC
                    xt_ps = pmisc.tile([128, BL], F32, tag="misc", name=f"xtps_{it}_{rep}")
                    for mb in range(NMB):
                        nc.tensor.matmul(xt_ps[0:FEAT + 1, mb * 128:(mb + 1) * 128],
                                         xchunks[c_idx][:, mb, slot, :], ident16[:, :],
                                         start=True, stop=True)
                    xT_next = xpool.tile([FEAT + 1, BL], MM_DT, tag="xT")
                    nc.vector.tensor_copy(xT_next[:, :], xt_ps[0:FEAT + 1, :])
                else:
                    xT_next = None

                # ---- gate tiles for L1(st1) and L0(it) ----
                # Emission order is phased so all chain-independent (free) MMs
                # precede all chain-gated MMs on the in-order PE queue:
                #   free:  hh1 starts for the first 3 L1 chunk-groups,
                #          ih0 starts for the f,i pairs
                #   gated: hh0(f,i)+acts, ih0(g,o)+hh0(g,o)+acts,
                #          ih1 finishes + remaining L1 groups
                a1 = None
                l1_jobs = []
                if 0 <= st1 < T:
                    a1 = {}
                    for g, fn in GATE_SEQ:
                        a1[g] = acts.tile([128, 2 * BL], F32 if g == 1 else FP16,
                                          tag=f"a1_{g}", name=f"a1_{g}_{it}_{rep}")
                    for g, fn in GATE_SEQ:
                        for half in range(2):
                            c = 2 * g + half
                            gt = pg1.tile([128, BL], F32, tag="g1",
                                          name=f"g1_{c}_{it}_{rep}")
                            l1_jobs.append((c, g, fn, gt))

                def l1_hh(job):
                    c, g, fn, gt = job
                    for kk in range(2):
                        nc.tensor.matmul(
                            gt[:], wt_hh1[:, (kk * 8 + c) * 128:(kk * 8 + c + 1) * 128],
                            h1_last[:, kk * BL:(kk + 1) * BL],
                            start=(kk == 0), stop=False)

                def l1_ih(job):
                    c, g, fn, gt = job
                    for kk in range(2):
                        nc.tensor.matmul(
                            gt[:], wt_ih1[:, (kk * 8 + c) * 128:(kk * 8 + c + 1) * 128],
                            h0_last[:, kk * BL:(kk + 1) * BL],
                            start=(st1 == 0 and kk == 0), stop=(kk == 1))
                    nc.scalar.activation(
                        a1[g][:, (c % 2) * BL:(c % 2 + 1) * BL],
                        gt[:], fn, bias=b1_sb[:, c:c + 1])

                NFREE = 3
                if l1_jobs and st1 > 0:
                    for job in l1_jobs[:NFREE]:
                        l1_hh(job)

                a0 = None
                gp0 = {}
                if it < T:
                    a0 = {}
                    for g, fn in GATE_SEQ:
                        a0[g] = acts.tile([128, 2 * BL], F32 if g == 1 else FP16,
                                          tag=f"a0_{g}", name=f"a0_{g}_{it}_{rep}")
                    for g, fn in GATE_SEQ:
                        gp0[g] = pg0.tile([128, 2 * BL], F32, tag="g0",
                                          name=f"g0_{g}_{it}_{rep}")

                def l0_ih(g):
                    gp = gp0[g]
                    for half in range(2):
                        c = 2 * g + half
                        nc.tensor.matmul(
                            gp[:, half * BL:(half + 1) * BL],
                            wt_ih0[:, c * 128:(c + 1) * 128],
                            xT[:], start=True, stop=(it == 0))

                def l0_hh_act(g, fn):
                    gp = gp0[g]
                    if it > 0:
                        for half in range(2):
                            c = 2 * g + half
                            for kk in range(2):
                                nc.tensor.matmul(
                                    gp[:, half * BL:(half + 1) * BL],
                                    wt_hh0[:, (kk * 8 + c) * 128:(kk * 8 + c + 1) * 128],
                                    h0_last[:, kk * BL:(kk + 1) * BL],
                                    start=False, stop=(kk == 1))
                    if g == 3:
                        for half in range(2):
                            nc.scalar.activation(
                                a0[g][:, half * BL:(half + 1) * BL],
                                gp[:, half * BL:(half + 1) * BL], fn)
                    else:
                        nc.scalar.activation(a0[g][:], gp[:], fn)

                if it < T:
                    # free starts for the early pair slots (f, i)
                    l0_ih(GATE_SEQ[0][0])
                    l0_ih(GATE_SEQ[1][0])
                    # gated work in dependency-arrival order
                    l0_hh_act(GATE_SEQ[0][0], GATE_SEQ[0][1])
                    l0_hh_act(GATE_SEQ[1][0], GATE_SEQ[1][1])
                    for g, fn in GATE_SEQ[2:]:
                        l0_ih(g)
                        l0_hh_act(g, fn)

                if l1_jobs:
                    for i, job in enumerate(l1_jobs):
                        if st1 > 0 and i >= NFREE:
                            l1_hh(job)
                        l1_ih(job)

                # ---- layer-0 elementwise chain -> h0(it), c0(it) ----
                if it < T:
                    c0_new = states.tile([128, 2 * BL], F32, tag="c0T")
                    ig0 = tmp.tile([128, 2 * BL], FP16, tag="tmp16")
                    nc.vector.tensor_mul(ig0[:], a0[0][:], a0[2][:])
                    if it > 0:
                        fc0 = tmp.tile([128, 2 * BL], F32, tag="tmp32")
                        nc.gpsimd.tensor_mul(fc0[:], a0[1][:], c0_last[:])
                    tc0 = tmp.tile([128, 2 * BL], FP16, tag="tmp16b")
                    h0_new = states.tile([128, 2 * BL], MM_DT, tag="h0T", bufs=3)
                    for kk in range(2):
                        r = slice(kk * BL, (kk + 1) * BL)
                        if it == 0:
                            nc.vector.tensor_copy(c0_new[:, r], ig0[:, r])
                        else:
                            nc.vector.tensor_add(c0_new[:, r], ig0[:, r], fc0[:, r])
                        nc.scalar.activation(tc0[:, r], c0_new[:, r], AF.Tanh)
                        nc.vector.tensor_mul(h0_new[:, r], a0[3][:, r], tc0[:, r])

                # ---- layer-1 elementwise chain -> h1(st1), c1(st1) ----
                if 0 <= st1 < T:
                    c1_new = states.tile([128, 2 * BL], F32, tag="c1T")
                    ig1 = tmp.tile([128, 2 * BL], FP16, tag="tmp16")
                    nc.vector.tensor_mul(ig1[:], a1[0][:], a1[2][:])
                    if st1 > 0:
                        fc1 = tmp.tile([128, 2 * BL], F32, tag="tmp32")
                        nc.gpsimd.tensor_mul(fc1[:], a1[1][:], c1_last[:])
                    tc1 = tmp.tile([128, 2 * BL], FP16, tag="tmp16b")
                    h1_new = states.tile([128, 2 * BL], MM_DT, tag="h1T", bufs=3)
                    for kk in range(2):
                        r = slice(kk * BL, (kk + 1) * BL)
                        if st1 == 0:
                            nc.vector.tensor_copy(c1_new[:, r], ig1[:, r])
                        else:
                            nc.vector.tensor_add(c1_new[:, r], ig1[:, r], fc1[:, r])
                        nc.scalar.activation(tc1[:, r], c1_new[:, r], AF.Tanh)
                        nc.vector.tensor_mul(h1_new[:, r], a1[3][:, r], tc1[:, r])

                # rebind pipeline registers
                if 0 <= st1 < T:
                    h1_hist[st1] = h1_new
                    h1_last, c1_last = h1_new, c1_new
                if it < T:
                    h0_last, c0_last = h0_new, c0_new
                xT = xT_next


B_FULL = 4096
N_CORES = 8

_nc_cache = {}


def _get_nc(T=MAXT, reps=1):
    key = (T, reps)
    if key not in _nc_cache:
        _nc_cache[key] = build_kernel(T=T, reps=reps)
    return _nc_cache[key]


def make_in_maps(inputs, n_cores=N_CORES):
    inp = {k: np.ascontiguousarray(np.asarray(v, dtype=np.float32))
           for k, v in inputs.items()}
    assert inp["z"].shape == (B_FULL, LATENT)
    maps = []
    for c in range(n_cores):
        sl = slice(c * BL, (c + 1) * BL)
        m = dict(inp)
        m["z"] = inp["z"][sl]
        m["target_seq"] = np.ascontiguousarray(inp["target_seq"][sl])
        maps.append(m)
    return maps


def kernel(**inputs) -> np.ndarray:
    from concourse.bass_utils import run_bass_kernel_spmd

    nc = _get_nc()
    maps = make_in_maps(inputs)
    res = run_bass_kernel_spmd(nc, maps, core_ids=list(range(N_CORES)))
    return np.concatenate([res.results[c]["out"] for c in range(N_CORES)], axis=0)

